# revision 28
# baseline (speedup 1.0000x reference)
"""FlowNet correlation kernel for Trainium2 (Bass/Tile), 8-core data-parallel.

out[b, j*21+i, y, x] = (1/C) * sum_c x1[b,c,y,x] * pad20(x2)[b,c, y+2j, x+2i]

Strategy (per core = one batch element):
  - Inputs are cast to bf16 on the host, output is bf16 on device and
    upcast on the host: halves every DRAM transfer.
  - Parity-split y and x (displacement stride 2); block pairs (y,x) into
    128-partition stationary tiles (RY=8 parity-rows x RX=16 parity-cols),
    pair index p = dy*RX + dx (dy-major).
  - PE computes the banded Gram rectangle per block in bf16:
    psum[pair, (a,b)] = <x1[:,pair], x2[:, halo(a,b)]>, halo 28x36.
  - Escape PSUM->SBUF bf16 with the 1/C scale (split DVE/ACT).
  - Per-pair 21x21 window gather bounces through DRAM (flat DRAM strides
    are unrestricted; SBUF partition steps must be whole rows): 8 slab
    writes/block with a dx-shear put pair q's window at base S*q + 36j+i.
    The read back fetches whole 741-elem slabs (1482B runs, full DMA
    bandwidth); a strided on-chip copy compacts 36j+i -> 21j+i.
  - PE transpose (vs bf16 identity) flips E to [ji, pair] chunks.
  - Merge-copies interleave both parities into bf16 [ji, 16 rows, w] out
    tiles (one per gy covering 16 consecutive rows -> 5120B DMA runs).
  - Slab writes and the read back ride different HWDGE rings so the
    write->read dependency always gets a real semaphore.
"""

import numpy as np
import ml_dtypes

import concourse.bacc as bacc
import concourse.bass as bass
import concourse.mybir as mybir
import concourse.tile as tile
from concourse.bass_utils import run_bass_kernel_spmd
from concourse.masks import make_identity

F32 = mybir.dt.float32
BF16 = mybir.dt.bfloat16

C = 256
H = 96
W = 160
NB = 8
J = 21          # taps per axis
PAD = 20
RY = 8          # parity rows per block
RX = 16         # parity cols per block
JI = J * J      # 441
JIPAD = 448
CHW = 112       # fold chunk width (JIPAD // 4)
SLAB = J * (RX + PAD)   # 756 contiguous elems per pair slab
SPAN = 36 * (J - 1) + J  # 741: last used slab offset (36*20+20) + 1
# The two px-quads of a (gy,py) pair are row-interleaved in the rect
# tile ([row][px][col], 72 elems per row-pair): the slab of scratch
# index m = gx*128+p holds rect rows [dy, dy+21) of BOTH px quads as
# ONE contiguous 1512-elem run, written at base T*m - dx (the dx-shear),
# so window (j,i) of quad px lives at T*m + 72*j + 36*px + i
# (partition-uniform). One readback descriptor of 72*20+36+21 = 1497
# elems covers both windows. Halves BOTH legs' descriptor counts vs the
# one-quad-per-slab layout.
PSLAB = 2 * SLAB  # 1512: px-paired slab run
T = 1536          # scratch stride per pair-slab (>= PSLAB + 15)
PSPAN = 72 * (J - 1) + 36 + J  # 1497: used readback span
EW = 1536         # eraw row elems per (partition, gx)


def build_nc(h=H, w=W, n_cores=NB):
    hp, wp = h // 2, w // 2
    gys, gxs = hp // RY, wp // RX
    ah, bw = RY + PAD, RX + PAD       # halo extents (28, 36)
    rect = ah * bw                    # 1008
    hw = h * w

    nc = bacc.Bacc("TRN2", target_bir_lowering=False, debug=False,
                   num_devices=n_cores)
    x1d = nc.dram_tensor("input1", [C, h, w], BF16, kind="ExternalInput")
    x2d = nc.dram_tensor("input2", [C, h, w], BF16, kind="ExternalInput")
    # 448 channels so the one-DMA-per-gy store can use all 4x112 chunk
    # partitions; the host slices [:441].
    outd = nc.dram_tensor("out", [JIPAD, h, w], BF16, kind="ExternalOutput")

    hwdge = [nc.sync, nc.scalar]      # the two HWDGE rings

    with tile.TileContext(nc) as tc:
        with (
            tc.tile_pool(name="x2pool", bufs=1) as x2pool,
            tc.tile_pool(name="x1pool", bufs=2) as x1pool,
            tc.tile_pool(name="identpool", bufs=1) as identpool,
            tc.tile_pool(name="rectpool", bufs=3) as rectpool,
            tc.tile_pool(name="epool", bufs=3) as epool,
            tc.tile_pool(name="erawpool", bufs=2) as erawpool,
            tc.tile_pool(name="outpool", bufs=1) as outpool,
            tc.tile_pool(name="dramscr", bufs=8, space="DRAM") as dramscr,
            tc.tile_pool(name="rectps", bufs=3, space="PSUM") as rectps,
            tc.tile_pool(name="foldps", bufs=2, space="PSUM") as foldps,
        ):
            ident = identpool.tile([128, 128], BF16)
            make_identity(nc, ident[:])

            # engines for the rect edge memsets, round-robin
            ms_engines = [nc.gpsimd, nc.vector]
            ms_idx = [0]

            def edge_memset(dst):
                ms_engines[ms_idx[0] % 2].memset(dst, 0.0)
                ms_idx[0] += 1

            # x2 in row-halves, low rows of both K-chunks first, so gy=0
            # matmuls (rows <= 37) start before the full 7.9MB lands.
            x2sb = x2pool.tile([128, 2, h, w], BF16)
            for half in range(2):
                r0 = half * (h // 2)
                r1 = r0 + h // 2
                for k in range(2):
                    hwdge[1].dma_start(
                        out=x2sb[:, k, r0:r1],
                        in_=x2d[k * 128:(k + 1) * 128, r0:r1])

            def load_x1(gy):
                # x1 rows for this group: one SWDGE load per K-chunk, then
                # a rearrange into block-major stationary tiles (walrus:
                # matmul weights APs must have ONE free dim). SWDGE keeps
                # the input stream off the two HWDGE rings. One 3D-AP
                # copy per (k, py, px) — (gx, ry, rx) folded into the AP.
                yb = 2 * RY * gy
                x1t = x1pool.tile([128, 2, 2 * RY, w], BF16, tag="x1t",
                                  bufs=1)
                for k in range(2):
                    nc.gpsimd.dma_start(
                        out=x1t[:, k],
                        in_=x1d[k * 128:(k + 1) * 128, yb:yb + 2 * RY])
                x1s = x1pool.tile([128, 2, 2, 2, gxs, RY * RX], BF16,
                                  tag="x1s", name=f"x1s{gy}")
                for k in range(2):
                    for py in range(2):
                        for px in range(2):
                            src = x1t[:, k, py::2, px::2].rearrange(
                                "p a (g b) -> p g a b", g=gxs)
                            dst = x1s[:, k, py, px].rearrange(
                                "p g (a b) -> p g a b", a=RY)
                            if k == 0:
                                nc.vector.tensor_copy(out=dst, in_=src)
                            else:
                                nc.scalar.copy(out=dst, in_=src)
                return x1s

            prow = gxs * ah * 2 * bw      # rs3 per-partition elems (10080)
            grow = ah * 2 * bw            # one gx sub-rect (2016)

            def pair_memsets(rs3, gy, py):
                # zero-fill clipped halo strips for BOTH px quads (the
                # parity split makes blo/bhi px-independent)
                y0 = py + 2 * RY * gy
                alo = max(0, -(-(PAD - y0) // 2))
                ahi = min(ah, (h - 1 - y0 + PAD) // 2 + 1)
                for gx in range(gxs):
                    x0 = 2 * RX * gx
                    blo = max(0, -(-(PAD - x0) // 2))
                    bhi = min(bw, (w - 1 - x0 + PAD) // 2 + 1)
                    if alo > 0:
                        edge_memset(rs3[:, gx, :alo])
                    if ahi < ah:
                        edge_memset(rs3[:, gx, ahi:])
                    for px in range(2):
                        if blo > 0:
                            edge_memset(rs3[:, gx, alo:ahi, px, :blo])
                        if bhi < bw:
                            edge_memset(rs3[:, gx, alo:ahi, px, bhi:])
                return alo, ahi

            def front_quad(x1s, gy, py, px, rs3, alo, ahi):
                """Matmuls + escapes into the px-interleaved rect."""
                y0 = py + 2 * RY * gy            # first real y row (parity)
                for gx in range(gxs):
                    x0 = px + 2 * RX * gx
                    # valid halo ranges (rows r = y0 + 2a - 20,
                    # cols u = x0 + 2b - 20)
                    blo = max(0, -(-(PAD - x0) // 2))
                    bhi = min(bw, (w - 1 - x0 + PAD) // 2 + 1)
                    nb_ = bhi - blo

                    # psum rect in two bank-aligned halves: half hh
                    # holds a in [14hh, 14hh+14) at [512hh, ...)
                    rp = rectps.tile([128, 2, 512], F32, tag="rp")

                    # banded Gram matmuls, K=256 in two 128-chunks,
                    # one matmul per psum-bank half per K-chunk
                    rpap = rp[:]
                    hranges = []
                    for hh in range(2):
                        a0 = max(alo, 14 * hh)
                        a1 = min(ahi, 14 * (hh + 1))
                        if a0 >= a1:
                            continue
                        hranges.append((hh, a0, a1))
                        na = a1 - a0
                        pout = bass.AP(
                            tensor=rpap.tensor,
                            offset=rpap.offset + 512 * hh,
                            ap=[[1024, 128], [1, na * nb_]])
                        for k in range(2):
                            lhsT = x1s[:, k, py, px, gx]
                            r0 = y0 + 2 * a0 - PAD
                            u0 = x0 + 2 * blo - PAD
                            rhs = x2sb[:, k,
                                       r0:r0 + 2 * na - 1:2,
                                       u0:u0 + 2 * nb_ - 1:2]
                            nc.tensor.matmul(
                                pout, lhsT, rhs,
                                start=(k == 0), stop=(k == 1))

                    # escape PSUM -> SBUF bf16 with 1/C scale
                    # (half 0 on DVE, half 1 on ACT)
                    for hh, a0, a1 in hranges:
                        na = a1 - a0
                        psrc = bass.AP(
                            tensor=rpap.tensor,
                            offset=rpap.offset + 512 * hh,
                            ap=[[1024, 128], [nb_, na], [1, nb_]])
                        if hh == 0:
                            nc.vector.tensor_scalar_mul(
                                rs3[:, gx, a0:a1, px, blo:bhi],
                                psrc, 1.0 / C)
                        else:
                            nc.scalar.mul(
                                rs3[:, gx, a0:a1, px, blo:bhi],
                                psrc, 1.0 / C)

            def slab_writes(eng, rs3, scrap):
                # gather leg 1 (8 DMAs): per dy-group g, partitions
                # [16g, 16g+16) share slab rows [g, g+21); the pair-slab
                # of scratch index m = gx*128+p (1512 elems: 21 rows x
                # both px) goes to base T*m - dx.
                rsap = rs3[:]
                for g in range(RY):
                    ssrc = bass.AP(
                        tensor=rsap.tensor,
                        offset=rsap.offset + RX * g * prow + 72 * g,
                        ap=[[prow, RX], [grow, gxs], [1, PSLAB]])
                    sdst = bass.AP(
                        tensor=scrap.tensor,
                        offset=scrap.offset + T * RX * g,
                        ap=[[T - 1, RX], [128 * T, gxs], [1, PSLAB]])
                    eng.dma_start(out=sdst, in_=ssrc)

            def readback(eng2, scrap):
                # Leg 2: ONE DMA on the other ring; each (p, gx)
                # descriptor spans PSPAN elems = both px windows.
                eraw = erawpool.tile([128, gxs, EW], BF16, tag="eraw")
                gsrc = bass.AP(
                    tensor=scrap.tensor,
                    offset=scrap.offset,
                    ap=[[T, 128], [128 * T, gxs], [1, PSPAN]])
                gdst = bass.AP(
                    tensor=eraw[:].tensor,
                    offset=eraw[:].offset,
                    ap=[[gxs * EW, 128], [EW, gxs], [1, PSPAN]])
                eng2.dma_start(out=gdst, in_=gsrc)
                return eraw

            def back_pair(eraw, ot, gy, py):
                erap = eraw[:]
                yb = 2 * RY * gy
                for px in range(2):
                    for gx in range(gxs):
                        x0 = px + 2 * RX * gx
                        # on-chip window compaction 36j+i -> 21j+i
                        et = epool.tile([128, JIPAD], BF16, tag="et")
                        nc.vector.memset(et[:, JI:], 0.0)
                        csrc = bass.AP(
                            tensor=erap.tensor,
                            offset=erap.offset + gx * EW + 36 * px,
                            ap=[[gxs * EW, 128], [72, J], [1, J]])
                        nc.vector.tensor_copy(
                            out=et[:, :JI].rearrange(
                                "p (j i) -> p j i", j=J),
                            in_=csrc)

                        # PE transpose chunks (all 4 share one PSUM
                        # bank) + merge into the gy out tile
                        # (pairs dy-major: free dims (dy RX, dx 1));
                        # y = py + 2dy within the 16-row tile, x = px
                        # + 2(16gx + dx): merges split DVE/ACT
                        fp = foldps.tile([CHW, 4, 128], BF16, tag="fp")
                        for ci in range(4):
                            nj = min(CHW, JI - ci * CHW)
                            nc.tensor.transpose(
                                fp[:, ci],
                                et[:, ci * CHW:(ci + 1) * CHW],
                                ident[:])
                            fpap = fp[:]
                            msrc = bass.AP(
                                tensor=fpap.tensor,
                                offset=fpap.offset + ci * 128,
                                ap=[[4 * 128, nj], [RX, RY], [1, RX]])
                            otap = ot[:]
                            mdst = bass.AP(
                                tensor=otap.tensor,
                                offset=otap.offset
                                + ci * 2 * RY * w + py * w + x0,
                                ap=[[4 * 2 * RY * w, nj], [2 * w, RY],
                                    [2, RX]])
                            if ci % 2 == 0:
                                nc.vector.tensor_copy(out=mdst,
                                                      in_=msrc)
                            else:
                                nc.scalar.copy(out=mdst, in_=msrc)

                if py == 1:
                    # DMA out via SWDGE, one chunk per DMA so the bursts
                    # interleave with bounce traffic; 16 consecutive rows
                    # per channel (5120B runs); channels [441,448) are
                    # junk the host drops.
                    for cih in range(4):
                        dst = bass.AP(
                            tensor=outd,
                            offset=cih * CHW * hw + yb * w,
                            ap=[[hw, CHW], [1, 2 * RY * w]])
                        nc.gpsimd.dma_start(out=dst, in_=ot[:, cih])

            # software pipeline over (gy, py) pairs, depth 2: pair k's
            # back half (compact -> transpose -> merge) is issued after
            # pair k+2's front half. The in-order Tensor queue then
            # holds [Grams k+2][transposes k] -- by the time PE reaches
            # the transposes, readback k (issued two pairs ago) has
            # drained, so the bounce round trip never stalls PE. Same
            # for escapes vs merges on the V/S queues.
            pairs = [(gy, py) for gy in range(gys) for py in range(2)]
            x1s = {0: load_x1(0)}
            ots = {}
            pending = []
            for pi, (gy, py) in enumerate(pairs):
                if py == 0:
                    ots[gy] = outpool.tile([CHW, 4, 2 * RY, w], BF16,
                                           tag="ot", name=f"ot{gy}")
                    if gy + 1 < gys:
                        x1s[gy + 1] = load_x1(gy + 1)
                rs3 = rectpool.tile([128, gxs, ah, 2, bw], BF16,
                                    tag="rs")
                alo, ahi = pair_memsets(rs3, gy, py)
                for px in range(2):
                    front_quad(x1s[gy], gy, py, px, rs3, alo, ahi)
                scr = dramscr.tile([T * 128 * gxs], BF16, tag="scr")
                scrap = scr[:]
                slab_writes(hwdge[pi % 2], rs3, scrap)
                eraw = readback(hwdge[(pi + 1) % 2], scrap)
                pending.append((eraw, ots[gy], gy, py))
                if len(pending) > 2:
                    back_pair(*pending.pop(0))
                if py == 1 and gy - 1 in x1s:
                    del x1s[gy - 1]
            for args in pending:
                back_pair(*args)

    nc.compile()
    return nc


_NC_CACHE = {}


def _get_nc(h, w, n_cores):
    key = (h, w, n_cores)
    if key not in _NC_CACHE:
        _NC_CACHE[key] = build_nc(h, w, n_cores)
    return _NC_CACHE[key]


def kernel(input1, input2):
    input1 = np.asarray(input1)
    input2 = np.asarray(input2)
    b, c, h, w = input1.shape
    assert c == C
    nc = _get_nc(h, w, b)
    bf = ml_dtypes.bfloat16
    in_maps = [
        {"input1": np.ascontiguousarray(input1[i]).astype(bf),
         "input2": np.ascontiguousarray(input2[i]).astype(bf)}
        for i in range(b)
    ]
    res = run_bass_kernel_spmd(nc, in_maps, core_ids=list(range(b)))
    return np.stack([res.results[i]["out"][:JI].astype(np.float32)
                     for i in range(b)])



# revision 30
# speedup vs baseline: 1.0845x; 1.0845x over previous
"""FlowNet correlation kernel for Trainium2 (Bass/Tile), 8-core data-parallel.

out[b, j*21+i, y, x] = (1/C) * sum_c x1[b,c,y,x] * pad20(x2)[b,c, y+2j, x+2i]

Strategy (per core = one batch element):
  - Inputs are cast to bf16 on the host, output is bf16 on device and
    upcast on the host: halves every DRAM transfer.
  - Parity-split y and x (displacement stride 2); block pairs (y,x) into
    128-partition stationary tiles (RY=8 parity-rows x RX=16 parity-cols),
    pair index p = dy*RX + dx (dy-major).
  - PE computes the banded Gram rectangle per block in bf16:
    psum[pair, (a,b)] = <x1[:,pair], x2[:, halo(a,b)]>, halo 28x36.
  - Escape PSUM->SBUF bf16 with the 1/C scale (split DVE/ACT).
  - Per-pair 21x21 window gather bounces through DRAM (flat DRAM strides
    are unrestricted; SBUF partition steps must be whole rows): 8 slab
    writes/block with a dx-shear put pair q's window at base S*q + 36j+i.
    The read back fetches whole 741-elem slabs (1482B runs, full DMA
    bandwidth); a strided on-chip copy compacts 36j+i -> 21j+i.
  - PE transpose (vs bf16 identity) flips E to [ji, pair] chunks.
  - Merge-copies interleave both parities into bf16 [ji, 16 rows, w] out
    tiles (one per gy covering 16 consecutive rows -> 5120B DMA runs).
  - Slab writes and the read back ride different HWDGE rings so the
    write->read dependency always gets a real semaphore.
"""

import numpy as np
import ml_dtypes

import concourse.bacc as bacc
import concourse.bass as bass
import concourse.mybir as mybir
import concourse.tile as tile
from concourse.bass_utils import run_bass_kernel_spmd
from concourse.masks import make_identity

F32 = mybir.dt.float32
BF16 = mybir.dt.bfloat16

C = 256
H = 96
W = 160
NB = 8
J = 21          # taps per axis
PAD = 20
RY = 8          # parity rows per block
RX = 16         # parity cols per block
JI = J * J      # 441
JIPAD = 448
CHW = 112       # fold chunk width (JIPAD // 4)
SLAB = J * (RX + PAD)   # 756 contiguous elems per pair slab
SPAN = 36 * (J - 1) + J  # 741: last used slab offset (36*20+20) + 1
# The two px-quads of a (gy,py) pair are row-interleaved in the rect
# tile ([row][px][col], 72 elems per row-pair): the slab of scratch
# index m = gx*128+p holds rect rows [dy, dy+21) of BOTH px quads as
# ONE contiguous 1512-elem run, written at base T*m - dx (the dx-shear),
# so window (j,i) of quad px lives at T*m + 72*j + 36*px + i
# (partition-uniform). One readback descriptor of 72*20+36+21 = 1497
# elems covers both windows. Halves BOTH legs' descriptor counts vs the
# one-quad-per-slab layout.
PSLAB = 2 * SLAB  # 1512: px-paired slab run
T = 1536          # scratch stride per pair-slab (>= PSLAB + 15)
PSPAN = 72 * (J - 1) + 36 + J  # 1497: used readback span
EW = 1536         # eraw row elems per (partition, gx)


def build_nc(h=H, w=W, n_cores=NB):
    hp, wp = h // 2, w // 2
    gys, gxs = hp // RY, wp // RX
    ah, bw = RY + PAD, RX + PAD       # halo extents (28, 36)
    rect = ah * bw                    # 1008
    hw = h * w

    nc = bacc.Bacc("TRN2", target_bir_lowering=False, debug=False,
                   num_devices=n_cores)
    x1d = nc.dram_tensor("input1", [C, h, w], BF16, kind="ExternalInput")
    x2d = nc.dram_tensor("input2", [C, h, w], BF16, kind="ExternalInput")
    # 448 channels so the one-DMA-per-gy store can use all 4x112 chunk
    # partitions; the host slices [:441].
    outd = nc.dram_tensor("out", [JIPAD, h, w], BF16, kind="ExternalOutput")

    hwdge = [nc.sync, nc.scalar]      # the two HWDGE rings

    with tile.TileContext(nc) as tc:
        with (
            tc.tile_pool(name="x2pool", bufs=1) as x2pool,
            tc.tile_pool(name="x1pool", bufs=2) as x1pool,
            tc.tile_pool(name="identpool", bufs=1) as identpool,
            tc.tile_pool(name="rectpool", bufs=2) as rectpool,
            tc.tile_pool(name="epool", bufs=3) as epool,
            tc.tile_pool(name="erawpool", bufs=2) as erawpool,
            tc.tile_pool(name="outpool", bufs=2) as outpool,
            tc.tile_pool(name="dramscr", bufs=8, space="DRAM") as dramscr,
            tc.tile_pool(name="rectps", bufs=3, space="PSUM") as rectps,
            tc.tile_pool(name="foldps", bufs=2, space="PSUM") as foldps,
        ):
            ident = identpool.tile([128, 128], BF16)
            make_identity(nc, ident[:])

            # engines for the rect edge memsets, round-robin
            ms_engines = [nc.gpsimd, nc.vector]
            ms_idx = [0]

            def edge_memset(dst):
                ms_engines[ms_idx[0] % 2].memset(dst, 0.0)
                ms_idx[0] += 1

            # x2 in row-halves, low rows of both K-chunks first, so gy=0
            # matmuls (rows <= 37) start before the full 7.9MB lands.
            x2sb = x2pool.tile([128, 2, h, w], BF16)
            for half in range(2):
                r0 = half * (h // 2)
                r1 = r0 + h // 2
                for k in range(2):
                    hwdge[1].dma_start(
                        out=x2sb[:, k, r0:r1],
                        in_=x2d[k * 128:(k + 1) * 128, r0:r1])

            def load_x1(gy):
                # x1 rows for this group: one SWDGE load per K-chunk, then
                # a rearrange into block-major stationary tiles (walrus:
                # matmul weights APs must have ONE free dim). SWDGE keeps
                # the input stream off the two HWDGE rings. One 3D-AP
                # copy per (k, py, px) — (gx, ry, rx) folded into the AP.
                yb = 2 * RY * gy
                x1t = x1pool.tile([128, 2, 2 * RY, w], BF16, tag="x1t",
                                  bufs=1)
                for k in range(2):
                    nc.gpsimd.dma_start(
                        out=x1t[:, k],
                        in_=x1d[k * 128:(k + 1) * 128, yb:yb + 2 * RY])
                x1s = x1pool.tile([128, 2, 2, 2, gxs, RY * RX], BF16,
                                  tag="x1s", name=f"x1s{gy}")
                for k in range(2):
                    for py in range(2):
                        for px in range(2):
                            src = x1t[:, k, py::2, px::2].rearrange(
                                "p a (g b) -> p g a b", g=gxs)
                            dst = x1s[:, k, py, px].rearrange(
                                "p g (a b) -> p g a b", a=RY)
                            if k == 0:
                                nc.vector.tensor_copy(out=dst, in_=src)
                            else:
                                nc.scalar.copy(out=dst, in_=src)
                return x1s

            prow = gxs * ah * 2 * bw      # rs3 per-partition elems (10080)
            grow = ah * 2 * bw            # one gx sub-rect (2016)

            def pair_memsets(rs3, gy, py):
                # zero-fill clipped halo strips for BOTH px quads (the
                # parity split makes blo/bhi px-independent)
                y0 = py + 2 * RY * gy
                alo = max(0, -(-(PAD - y0) // 2))
                ahi = min(ah, (h - 1 - y0 + PAD) // 2 + 1)
                for gx in range(gxs):
                    x0 = 2 * RX * gx
                    blo = max(0, -(-(PAD - x0) // 2))
                    bhi = min(bw, (w - 1 - x0 + PAD) // 2 + 1)
                    if alo > 0:
                        edge_memset(rs3[:, gx, :alo])
                    if ahi < ah:
                        edge_memset(rs3[:, gx, ahi:])
                    for px in range(2):
                        if blo > 0:
                            edge_memset(rs3[:, gx, alo:ahi, px, :blo])
                        if bhi < bw:
                            edge_memset(rs3[:, gx, alo:ahi, px, bhi:])
                return alo, ahi

            def front_quad(x1s, gy, py, px, rs3, alo, ahi):
                """Matmuls + escapes into the px-interleaved rect."""
                y0 = py + 2 * RY * gy            # first real y row (parity)
                for gx in range(gxs):
                    x0 = px + 2 * RX * gx
                    # valid halo ranges (rows r = y0 + 2a - 20,
                    # cols u = x0 + 2b - 20)
                    blo = max(0, -(-(PAD - x0) // 2))
                    bhi = min(bw, (w - 1 - x0 + PAD) // 2 + 1)
                    nb_ = bhi - blo

                    # psum rect in two bank-aligned halves: half hh
                    # holds a in [14hh, 14hh+14) at [512hh, ...)
                    rp = rectps.tile([128, 2, 512], F32, tag="rp")

                    # banded Gram matmuls, K=256 in two 128-chunks,
                    # one matmul per psum-bank half per K-chunk
                    rpap = rp[:]
                    hranges = []
                    for hh in range(2):
                        a0 = max(alo, 14 * hh)
                        a1 = min(ahi, 14 * (hh + 1))
                        if a0 >= a1:
                            continue
                        hranges.append((hh, a0, a1))
                        na = a1 - a0
                        pout = bass.AP(
                            tensor=rpap.tensor,
                            offset=rpap.offset + 512 * hh,
                            ap=[[1024, 128], [1, na * nb_]])
                        for k in range(2):
                            lhsT = x1s[:, k, py, px, gx]
                            r0 = y0 + 2 * a0 - PAD
                            u0 = x0 + 2 * blo - PAD
                            rhs = x2sb[:, k,
                                       r0:r0 + 2 * na - 1:2,
                                       u0:u0 + 2 * nb_ - 1:2]
                            nc.tensor.matmul(
                                pout, lhsT, rhs,
                                start=(k == 0), stop=(k == 1))

                    # escape PSUM -> SBUF bf16 with 1/C scale
                    # (half 0 on DVE, half 1 on ACT)
                    for hh, a0, a1 in hranges:
                        na = a1 - a0
                        psrc = bass.AP(
                            tensor=rpap.tensor,
                            offset=rpap.offset + 512 * hh,
                            ap=[[1024, 128], [nb_, na], [1, nb_]])
                        if hh == 0:
                            nc.vector.tensor_scalar_mul(
                                rs3[:, gx, a0:a1, px, blo:bhi],
                                psrc, 1.0 / C)
                        else:
                            nc.scalar.mul(
                                rs3[:, gx, a0:a1, px, blo:bhi],
                                psrc, 1.0 / C)

            def slab_writes(eng, rs3, scrap):
                # gather leg 1 (8 DMAs): per dy-group g, partitions
                # [16g, 16g+16) share slab rows [g, g+21); the pair-slab
                # of scratch index m = gx*128+p (1512 elems: 21 rows x
                # both px) goes to base T*m - dx.
                rsap = rs3[:]
                for g in range(RY):
                    ssrc = bass.AP(
                        tensor=rsap.tensor,
                        offset=rsap.offset + RX * g * prow + 72 * g,
                        ap=[[prow, RX], [grow, gxs], [1, PSLAB]])
                    sdst = bass.AP(
                        tensor=scrap.tensor,
                        offset=scrap.offset + T * RX * g,
                        ap=[[T - 1, RX], [128 * T, gxs], [1, PSLAB]])
                    eng.dma_start(out=sdst, in_=ssrc)

            def readback(eng2, scrap):
                # Leg 2: ONE DMA on the other ring; each (p, gx)
                # descriptor spans PSPAN elems = both px windows.
                eraw = erawpool.tile([128, gxs, EW], BF16, tag="eraw")
                gsrc = bass.AP(
                    tensor=scrap.tensor,
                    offset=scrap.offset,
                    ap=[[T, 128], [128 * T, gxs], [1, PSPAN]])
                gdst = bass.AP(
                    tensor=eraw[:].tensor,
                    offset=eraw[:].offset,
                    ap=[[gxs * EW, 128], [EW, gxs], [1, PSPAN]])
                eng2.dma_start(out=gdst, in_=gsrc)
                return eraw

            def back_pair(eraw, ot, gy, py):
                erap = eraw[:]
                yb = 2 * RY * gy
                for px in range(2):
                    for gx in range(gxs):
                        x0 = px + 2 * RX * gx
                        # on-chip window compaction 36j+i -> 21j+i
                        et = epool.tile([128, JIPAD], BF16, tag="et")
                        nc.vector.memset(et[:, JI:], 0.0)
                        csrc = bass.AP(
                            tensor=erap.tensor,
                            offset=erap.offset + gx * EW + 36 * px,
                            ap=[[gxs * EW, 128], [72, J], [1, J]])
                        nc.vector.tensor_copy(
                            out=et[:, :JI].rearrange(
                                "p (j i) -> p j i", j=J),
                            in_=csrc)

                        # PE transpose chunks (all 4 share one PSUM
                        # bank) + merge into the gy out tile
                        # (pairs dy-major: free dims (dy RX, dx 1));
                        # y = py + 2dy within the 16-row tile, x = px
                        # + 2(16gx + dx): merges split DVE/ACT
                        fp = foldps.tile([CHW, 4, 128], BF16, tag="fp")
                        for ci in range(4):
                            nj = min(CHW, JI - ci * CHW)
                            nc.tensor.transpose(
                                fp[:, ci],
                                et[:, ci * CHW:(ci + 1) * CHW],
                                ident[:])
                            fpap = fp[:]
                            msrc = bass.AP(
                                tensor=fpap.tensor,
                                offset=fpap.offset + ci * 128,
                                ap=[[4 * 128, nj], [RX, RY], [1, RX]])
                            otap = ot[:]
                            mdst = bass.AP(
                                tensor=otap.tensor,
                                offset=otap.offset
                                + ci * 2 * RY * w + py * w + x0,
                                ap=[[4 * 2 * RY * w, nj], [2 * w, RY],
                                    [2, RX]])
                            if ci % 2 == 0:
                                nc.vector.tensor_copy(out=mdst,
                                                      in_=msrc)
                            else:
                                nc.scalar.copy(out=mdst, in_=msrc)

                if py == 1:
                    # DMA out via SWDGE, one chunk per DMA so the bursts
                    # interleave with bounce traffic; 16 consecutive rows
                    # per channel (5120B runs); channels [441,448) are
                    # junk the host drops.
                    for cih in range(4):
                        dst = bass.AP(
                            tensor=outd,
                            offset=cih * CHW * hw + yb * w,
                            ap=[[hw, CHW], [1, 2 * RY * w]])
                        nc.gpsimd.dma_start(out=dst, in_=ot[:, cih])

            # software pipeline over (gy, py) pairs, depth 2: pair k's
            # back half (compact -> transpose -> merge) is issued after
            # pair k+2's front half. The in-order Tensor queue then
            # holds [Grams k+2][transposes k] -- by the time PE reaches
            # the transposes, readback k (issued two pairs ago) has
            # drained, so the bounce round trip never stalls PE. Same
            # for escapes vs merges on the V/S queues.
            pairs = [(gy, py) for gy in range(gys) for py in range(2)]
            x1s = {0: load_x1(0)}
            ots = {}
            pending = []
            for pi, (gy, py) in enumerate(pairs):
                if py == 0:
                    ots[gy] = outpool.tile([CHW, 4, 2 * RY, w], BF16,
                                           tag="ot", name=f"ot{gy}")
                    if gy + 1 < gys:
                        x1s[gy + 1] = load_x1(gy + 1)
                rs3 = rectpool.tile([128, gxs, ah, 2, bw], BF16,
                                    tag="rs")
                alo, ahi = pair_memsets(rs3, gy, py)
                for px in range(2):
                    front_quad(x1s[gy], gy, py, px, rs3, alo, ahi)
                scr = dramscr.tile([T * 128 * gxs], BF16, tag="scr")
                scrap = scr[:]
                slab_writes(hwdge[pi % 2], rs3, scrap)
                eraw = readback(hwdge[(pi + 1) % 2], scrap)
                pending.append((eraw, ots[gy], gy, py))
                if len(pending) > 2:
                    back_pair(*pending.pop(0))
                if py == 1 and gy - 1 in x1s:
                    del x1s[gy - 1]
            for args in pending:
                back_pair(*args)

    nc.compile()
    return nc


_NC_CACHE = {}


def _get_nc(h, w, n_cores):
    key = (h, w, n_cores)
    if key not in _NC_CACHE:
        _NC_CACHE[key] = build_nc(h, w, n_cores)
    return _NC_CACHE[key]


def kernel(input1, input2):
    input1 = np.asarray(input1)
    input2 = np.asarray(input2)
    b, c, h, w = input1.shape
    assert c == C
    nc = _get_nc(h, w, b)
    bf = ml_dtypes.bfloat16
    in_maps = [
        {"input1": np.ascontiguousarray(input1[i]).astype(bf),
         "input2": np.ascontiguousarray(input2[i]).astype(bf)}
        for i in range(b)
    ]
    res = run_bass_kernel_spmd(nc, in_maps, core_ids=list(range(b)))
    return np.stack([res.results[i]["out"][:JI].astype(np.float32)
                     for i in range(b)])

